# revision 16
# baseline (speedup 1.0000x reference)
"""CLUB loss kernel for 8x TRN2 NeuronCores.

Math: the reference computes, per sample b (L=512 positions, D=64 dims):
  mu     = MLP_mu(x);  logvar = tanh(MLP_lv(x));  iv = exp(-logvar)
  positive[d,l] = -(mu - y)^2 * 0.5 * iv
  negative[d,l] = -mean_j (y[d,j] - mu[d,l])^2 * 0.5 * iv
  loss = mean over (b,l) of sum_d (positive - negative)

The pairwise LxL mean collapses via moments of y over positions:
  mean_j (y_j - mu)^2 = Ey2 - 2*mu*Ey + mu^2
so with yd2 = 2*(y - Ey), ysq = y^2:
  loss = -0.5/(B*L) * sum_{b,d,l} [ ((ysq - Ey2) - mu*yd2) * iv ].

Sharding: data-parallel over batch B=8, one sample per core; each core
returns a [64,1] per-partition partial, host does the tiny final reduce.

Matmul operands are float32r (fp32 bits, single-pass PE mode, ~4x faster
than fp32 matmul); y, b2 biases and all elementwise math stay fp32.

Input packing per core:
  img [128, 1154] f32r:
    0:128    w1muT_a   = mu_W1.T rows 0:128
    128:256  w1lvT_a   = lv_W1.T rows 0:128
    256:384  w1muT_b   = mu_W1.T rows 128:192 in rows 64:128 (rows 0:64 pad)
    384:512  w1lvT_b   = lv_W1.T rows 128:192 in rows 64:128 (rows 0:64 pad)
    512:576  w2muT     = mu_W2.T
    576:640  w2lvT     = lv_W2.T
    640:642  b1mu, b1lv
    642:1154 xa        = x rows 0:128
  xb [64, 512] f32r: x rows 128:192 -> SBUF partitions 64:128 (aligned with
    the chunk-B weights; matmul requires matching base partitions)
  yb [64, 514] f32: y (cols 0:512), b2mu (col 512), b2lv (col 513)
"""

import sys

if "/opt/trn_rl_repo" not in sys.path:
    sys.path.insert(0, "/opt/trn_rl_repo")

import numpy as np

B, L = 8, 512
XD, YD, H = 192, 64, 128
NCORES = 8
IMGC = 1154
YBC = 514

_CACHE: dict = {}


def build_nc(debug: bool = False, img_splits: int = 2, yb_engine: str = "gpsimd"):
    import concourse.bass as bass
    import concourse.bacc as bacc
    import concourse.tile as tile
    from concourse import mybir

    f32 = mybir.dt.float32
    f32r = mybir.dt.float32r
    AF = mybir.ActivationFunctionType
    OP = mybir.AluOpType

    nc = bacc.Bacc("TRN2", target_bir_lowering=False, debug=debug)

    img_d = nc.dram_tensor("img", [128, IMGC], f32r, kind="ExternalInput")
    xb_d = nc.dram_tensor("xb", [64, L], f32r, kind="ExternalInput")
    yb_d = nc.dram_tensor("yb", [64, YBC], f32, kind="ExternalInput")
    acc_d = nc.dram_tensor("acc", [YD, 1], f32, kind="ExternalOutput")

    with tile.TileContext(nc) as tc:
        with (
            tc.tile_pool(name="sb", bufs=1) as sb,
            tc.tile_pool(name="ps", bufs=1, space=bass.MemorySpace.PSUM) as ps,
        ):
            img = sb.tile([128, IMGC], f32r, tag="img")
            # split the big image DMA across the two HWDGE rings (sync+scalar)
            dma_engines = [nc.sync, nc.scalar]
            bounds = [IMGC * i // img_splits for i in range(img_splits + 1)]
            for i in range(img_splits):
                lo, hi = bounds[i], bounds[i + 1]
                eng = dma_engines[i % len(dma_engines)]
                eng.dma_start(out=img[:, lo:hi], in_=img_d[:, lo:hi])
            xbr = sb.tile([128, L], f32r, tag="xbr")
            getattr(nc, yb_engine).dma_start(out=xbr[64:128, :], in_=xb_d[:, :])
            ybt = sb.tile([64, YBC], f32, tag="ybt")
            getattr(nc, yb_engine).dma_start(out=ybt, in_=yb_d[:, :])

            w1muT_a = img[:, 0:128]
            w1lvT_a = img[:, 128:256]
            w1muT_b = img[64:128, 256:384]
            w1lvT_b = img[64:128, 384:512]
            w2muT = img[:, 512:576]
            w2lvT = img[:, 576:640]
            b1mu = img[:, 640:641]
            b1lv = img[:, 641:642]
            xa = img[:, 642:1154]
            xb = xbr[64:128, :]
            y = ybt[:, 0:512]
            b2mu = ybt[:, 512:513]
            b2lv = ybt[:, 513:514]

            # --- moments of y over L ---
            sumy = sb.tile([64, 1], f32, tag="sumy")
            nc.vector.reduce_sum(out=sumy, in_=y, axis=mybir.AxisListType.X)
            ey = sb.tile([64, 1], f32, tag="ey")
            nc.scalar.mul(ey, sumy, 1.0 / L)
            ysq = sb.tile([64, L], f32, tag="ysq")
            sumy2 = sb.tile([64, 1], f32, tag="sumy2")
            nc.scalar.activation(out=ysq, in_=y, func=AF.Square, accum_out=sumy2)
            ey2 = sb.tile([64, 1], f32, tag="ey2")
            nc.scalar.mul(ey2, sumy2, 1.0 / L)

            # --- mu path ---
            h_mu = ps.tile([128, L], f32, tag="hmu")
            nc.tensor.matmul(h_mu, w1muT_a, xa, start=True, stop=False)
            nc.tensor.matmul(h_mu, w1muT_b, xb, start=False, stop=True)
            h_mu_s = sb.tile([128, L], f32r, tag="hmus")
            nc.scalar.activation(out=h_mu_s, in_=h_mu, func=AF.Relu, bias=b1mu, scale=1.0)
            mu_nb = ps.tile([64, L], f32, tag="munb")
            nc.tensor.matmul(mu_nb, w2muT, h_mu_s[:, :], start=True, stop=True)

            # --- lv path ---
            h_lv = ps.tile([128, L], f32, tag="hlv")
            nc.tensor.matmul(h_lv, w1lvT_a, xa, start=True, stop=False)
            nc.tensor.matmul(h_lv, w1lvT_b, xb, start=False, stop=True)
            h_lv_s = sb.tile([128, L], f32r, tag="hlvs")
            nc.scalar.activation(out=h_lv_s, in_=h_lv, func=AF.Relu, bias=b1lv, scale=1.0)
            lv_nb = ps.tile([64, L], f32, tag="lvnb")
            nc.tensor.matmul(lv_nb, w2lvT, h_lv_s[:, :], start=True, stop=True)
            t1 = sb.tile([64, L], f32, tag="t1")
            nc.scalar.activation(out=t1, in_=lv_nb, func=AF.Tanh, bias=b2lv, scale=1.0)
            iv = sb.tile([64, L], f32, tag="iv")
            nc.scalar.activation(out=iv, in_=t1, func=AF.Exp, scale=-1.0)

            # --- elementwise + reduction ---
            yd2 = sb.tile([64, L], f32, tag="yd2")
            nc.vector.tensor_scalar(
                out=yd2, in0=y, scalar1=ey, scalar2=2.0, op0=OP.subtract, op1=OP.mult
            )
            w = sb.tile([64, L], f32, tag="w")
            nc.vector.scalar_tensor_tensor(
                out=w, in0=mu_nb, scalar=b2mu, in1=yd2, op0=OP.add, op1=OP.mult
            )
            v = sb.tile([64, L], f32, tag="v")
            nc.vector.scalar_tensor_tensor(
                out=v, in0=ysq, scalar=ey2, in1=w, op0=OP.subtract, op1=OP.subtract
            )
            scr = sb.tile([64, L], f32, tag="scr")
            acc_s = sb.tile([64, 1], f32, tag="accs")
            nc.vector.scalar_tensor_tensor(
                out=scr, in0=v, scalar=1.0, in1=iv,
                op0=OP.mult, op1=OP.mult, accum_out=acc_s,
            )
            nc.sync.dma_start(out=acc_d[:, :], in_=acc_s)

    nc.compile()
    return nc


def pack_inputs(inputs: dict) -> list[dict]:
    x = np.ascontiguousarray(np.asarray(inputs["x_samples"], dtype=np.float32))
    y = np.ascontiguousarray(np.asarray(inputs["y_samples"], dtype=np.float32))
    mu_W1 = np.asarray(inputs["mu_W1"], dtype=np.float32)
    mu_b1 = np.asarray(inputs["mu_b1"], dtype=np.float32)
    mu_W2 = np.asarray(inputs["mu_W2"], dtype=np.float32)
    mu_b2 = np.asarray(inputs["mu_b2"], dtype=np.float32)
    lv_W1 = np.asarray(inputs["lv_W1"], dtype=np.float32)
    lv_b1 = np.asarray(inputs["lv_b1"], dtype=np.float32)
    lv_W2 = np.asarray(inputs["lv_W2"], dtype=np.float32)
    lv_b2 = np.asarray(inputs["lv_b2"], dtype=np.float32)

    img = np.zeros((128, IMGC), np.float32)
    w1muT = mu_W1.T  # [192, 128]
    w1lvT = lv_W1.T
    img[:, 0:128] = w1muT[0:128]
    img[:, 128:256] = w1lvT[0:128]
    img[64:128, 256:384] = w1muT[128:192]
    img[64:128, 384:512] = w1lvT[128:192]
    img[:, 512:576] = mu_W2.T
    img[:, 576:640] = lv_W2.T
    img[:, 640] = mu_b1
    img[:, 641] = lv_b1

    in_maps = []
    for b in range(NCORES):
        im = img.copy()
        im[:, 642:1154] = x[b, 0:128]
        yb = np.zeros((64, YBC), np.float32)
        yb[:, 0:512] = y[b]
        yb[:, 512] = mu_b2
        yb[:, 513] = lv_b2
        in_maps.append(
            {"img": im, "xb": np.ascontiguousarray(x[b, 128:192]), "yb": yb}
        )
    return in_maps


def kernel(**inputs) -> np.ndarray:
    from concourse.bass_utils import run_bass_kernel_spmd

    if "nc" not in _CACHE:
        _CACHE["nc"] = build_nc(debug=False)
    nc = _CACHE["nc"]

    in_maps = pack_inputs(inputs)
    res = run_bass_kernel_spmd(nc, in_maps, core_ids=list(range(NCORES)))
    tot = 0.0
    for r in res.results:
        tot += float(r["acc"].astype(np.float64).sum())
    loss = -0.5 * tot / (B * L)
    return np.array(loss, dtype=np.float32)


# revision 22
# speedup vs baseline: 1.2100x; 1.2100x over previous
"""CLUB loss kernel for 8x TRN2 NeuronCores.

Math: the reference computes, per sample b (L=512 positions, D=64 dims):
  mu     = MLP_mu(x);  logvar = tanh(MLP_lv(x));  iv = exp(-logvar)
  positive[d,l] = -(mu - y)^2 * 0.5 * iv
  negative[d,l] = -mean_j (y[d,j] - mu[d,l])^2 * 0.5 * iv
  loss = mean over (b,l) of sum_d (positive - negative)

The pairwise LxL mean collapses via moments of y over positions:
  mean_j (y_j - mu)^2 = Ey2 - 2*mu*Ey + mu^2
so with yd2 = 2*(y - Ey), ysq = y^2:
  loss = -0.5/(B*L) * sum_{b,d,l} [ ((ysq - Ey2) - mu*yd2) * iv ].

Sharding: data-parallel over batch B=8, one sample per core; each core
returns a [64,1] per-partition partial, host does the tiny final reduce.

Matmul operands are float32r (fp32 bits, single-pass PE mode, ~4x faster
than fp32 matmul); y, b2 biases and all elementwise math stay fp32.

Input packing per core:
  img [128, 1154] f32r:
    0:128    w1muT_a   = mu_W1.T rows 0:128
    128:256  w1lvT_a   = lv_W1.T rows 0:128
    256:384  w1muT_b   = mu_W1.T rows 128:192 in rows 64:128 (rows 0:64 pad)
    384:512  w1lvT_b   = lv_W1.T rows 128:192 in rows 64:128 (rows 0:64 pad)
    512:576  w2muT     = mu_W2.T
    576:640  w2lvT     = lv_W2.T
    640:642  b1mu, b1lv
    642:1154 xa        = x rows 0:128
  xb [64, 512] f32r: x rows 128:192 -> SBUF partitions 64:128 (aligned with
    the chunk-B weights; matmul requires matching base partitions)
  yb [64, 514] f32: y (cols 0:512), b2mu (col 512), b2lv (col 513)
"""

import sys

if "/opt/trn_rl_repo" not in sys.path:
    sys.path.insert(0, "/opt/trn_rl_repo")

import numpy as np

B, L = 8, 512
XD, YD, H = 192, 64, 128
NCORES = 8
IMGC = 1154
YBC = 514

_CACHE: dict = {}


def build_nc(debug: bool = False, img_splits: int = 2, yb_engine: str = "gpsimd"):
    import concourse.bass as bass
    import concourse.bacc as bacc
    import concourse.tile as tile
    from concourse import mybir

    f32 = mybir.dt.float32
    f32r = mybir.dt.float32r
    AF = mybir.ActivationFunctionType
    OP = mybir.AluOpType

    nc = bacc.Bacc("TRN2", target_bir_lowering=False, debug=debug)

    w1_d = nc.dram_tensor("w1", [128, 512], f32r, kind="ExternalInput")
    w2_d = nc.dram_tensor("w2", [128, 128], f32r, kind="ExternalInput")
    b1_d = nc.dram_tensor("b1", [128, 2], f32, kind="ExternalInput")
    xa_d = nc.dram_tensor("xa", [128, L], f32r, kind="ExternalInput")
    xb_d = nc.dram_tensor("xb", [64, L], f32r, kind="ExternalInput")
    yb_d = nc.dram_tensor("yb", [64, YBC], f32, kind="ExternalInput")
    acc_d = nc.dram_tensor("acc", [1, 1], f32, kind="ExternalOutput")

    with tile.TileContext(nc) as tc:
        with (
            tc.tile_pool(name="sb", bufs=1) as sb,
            tc.tile_pool(name="ps", bufs=1, space=bass.MemorySpace.PSUM) as ps,
        ):
            ones = sb.tile([64, 1], mybir.dt.float32, tag="ones")
            nc.gpsimd.memset(ones, 1.0)
            # ring A (sync): W1 pack, then xa. ring B (scalar): xb, yb, W2 pack.
            w1t = sb.tile([128, 512], f32r, tag="w1t")
            nc.sync.dma_start(out=w1t, in_=w1_d[:, :])
            xat = sb.tile([128, L], f32r, tag="xat")
            nc.sync.dma_start(out=xat, in_=xa_d[:, :])
            xbr = sb.tile([128, L], f32r, tag="xbr")
            nc.scalar.dma_start(out=xbr[64:128, :], in_=xb_d[:, :])
            ybt = sb.tile([64, YBC], f32, tag="ybt")
            nc.scalar.dma_start(out=ybt, in_=yb_d[:, :])
            w2t = sb.tile([128, 128], f32r, tag="w2t")
            nc.scalar.dma_start(out=w2t, in_=w2_d[:, :])
            b1t = sb.tile([128, 2], f32, tag="b1t")
            nc.scalar.dma_start(out=b1t, in_=b1_d[:, :])

            w1muT_a = w1t[:, 0:128]
            w1lvT_a = w1t[:, 128:256]
            w1muT_b = w1t[64:128, 256:384]
            w1lvT_b = w1t[64:128, 384:512]
            w2muT = w2t[:, 0:64]
            w2lvT = w2t[:, 64:128]
            b1mu = b1t[:, 0:1]
            b1lv = b1t[:, 1:2]
            xa = xat[:, :]
            xb = xbr[64:128, :]
            y = ybt[:, 0:512]
            b2mu = ybt[:, 512:513]
            b2lv = ybt[:, 513:514]

            # --- moments of y over L ---
            sumy = sb.tile([64, 1], f32, tag="sumy")
            nc.vector.reduce_sum(out=sumy, in_=y, axis=mybir.AxisListType.X)
            ey = sb.tile([64, 1], f32, tag="ey")
            nc.scalar.mul(ey, sumy, 1.0 / L)
            ysq = sb.tile([64, L], f32, tag="ysq")
            sumy2 = sb.tile([64, 1], f32, tag="sumy2")
            nc.scalar.activation(out=ysq, in_=y, func=AF.Square, accum_out=sumy2)
            ey2 = sb.tile([64, 1], f32, tag="ey2")
            nc.scalar.mul(ey2, sumy2, 1.0 / L)

            # --- mu path ---
            h_mu = ps.tile([128, L], f32, tag="hmu")
            nc.tensor.matmul(h_mu, w1muT_a, xa, start=True, stop=False)
            nc.tensor.matmul(h_mu, w1muT_b, xb, start=False, stop=True)
            h_mu_s = sb.tile([128, L], f32r, tag="hmus")
            nc.scalar.activation(out=h_mu_s, in_=h_mu, func=AF.Relu, bias=b1mu, scale=1.0)
            mu_nb = ps.tile([64, L], f32, tag="munb")
            nc.tensor.matmul(mu_nb, w2muT, h_mu_s[:, :], start=True, stop=True)

            # --- lv path ---
            h_lv = ps.tile([128, L], f32, tag="hlv")
            nc.tensor.matmul(h_lv, w1lvT_a, xa, start=True, stop=False)
            nc.tensor.matmul(h_lv, w1lvT_b, xb, start=False, stop=True)
            h_lv_s = sb.tile([128, L], f32r, tag="hlvs")
            nc.scalar.activation(out=h_lv_s, in_=h_lv, func=AF.Relu, bias=b1lv, scale=1.0)
            lv_nb = ps.tile([64, L], f32, tag="lvnb")
            nc.tensor.matmul(lv_nb, w2lvT, h_lv_s[:, :], start=True, stop=True)
            t1 = sb.tile([64, L], f32, tag="t1")
            nc.scalar.activation(out=t1, in_=lv_nb, func=AF.Tanh, bias=b2lv, scale=1.0)
            iv = sb.tile([64, L], f32, tag="iv")
            nc.scalar.activation(out=iv, in_=t1, func=AF.Exp, scale=-1.0)

            # --- elementwise + reduction ---
            yd2 = sb.tile([64, L], f32, tag="yd2")
            nc.vector.tensor_scalar(
                out=yd2, in0=y, scalar1=ey, scalar2=2.0, op0=OP.subtract, op1=OP.mult
            )
            w = sb.tile([64, L], f32, tag="w")
            nc.vector.scalar_tensor_tensor(
                out=w, in0=mu_nb, scalar=b2mu, in1=yd2, op0=OP.add, op1=OP.mult
            )
            v = sb.tile([64, L], f32, tag="v")
            nc.vector.scalar_tensor_tensor(
                out=v, in0=ysq, scalar=ey2, in1=w, op0=OP.subtract, op1=OP.subtract
            )
            scr = sb.tile([64, L], f32, tag="scr")
            acc_s = sb.tile([64, 1], f32, tag="accs")
            nc.vector.scalar_tensor_tensor(
                out=scr, in0=v, scalar=1.0, in1=iv,
                op0=OP.mult, op1=OP.mult, accum_out=acc_s,
            )
            # collapse the [64,1] per-partition partials to a single scalar on
            # the PE (ones.T @ acc), so the output DMA is one 4-byte packet —
            # a [64,1] store costs 64 tiny descriptors + 16 lazy sem incs.
            acc_ps = ps.tile([1, 1], f32, tag="accps")
            nc.tensor.matmul(acc_ps, acc_s, ones, start=True, stop=True)
            acc_sb = sb.tile([1, 1], f32, tag="accsb")
            nc.scalar.copy(acc_sb, acc_ps)
            nc.sync.dma_start(out=acc_d[:, :], in_=acc_sb, single_packet=True)

    nc.compile()
    return nc


def pack_inputs(inputs: dict) -> list[dict]:
    x = np.ascontiguousarray(np.asarray(inputs["x_samples"], dtype=np.float32))
    y = np.ascontiguousarray(np.asarray(inputs["y_samples"], dtype=np.float32))
    mu_W1 = np.asarray(inputs["mu_W1"], dtype=np.float32)
    mu_b1 = np.asarray(inputs["mu_b1"], dtype=np.float32)
    mu_W2 = np.asarray(inputs["mu_W2"], dtype=np.float32)
    mu_b2 = np.asarray(inputs["mu_b2"], dtype=np.float32)
    lv_W1 = np.asarray(inputs["lv_W1"], dtype=np.float32)
    lv_b1 = np.asarray(inputs["lv_b1"], dtype=np.float32)
    lv_W2 = np.asarray(inputs["lv_W2"], dtype=np.float32)
    lv_b2 = np.asarray(inputs["lv_b2"], dtype=np.float32)

    w1 = np.zeros((128, 512), np.float32)
    w1muT = mu_W1.T  # [192, 128]
    w1lvT = lv_W1.T
    w1[:, 0:128] = w1muT[0:128]
    w1[:, 128:256] = w1lvT[0:128]
    w1[64:128, 256:384] = w1muT[128:192]
    w1[64:128, 384:512] = w1lvT[128:192]
    w2 = np.concatenate([mu_W2.T, lv_W2.T], axis=1)  # [128, 128]
    b1 = np.stack([mu_b1, lv_b1], axis=1)  # [128, 2]

    in_maps = []
    for b in range(NCORES):
        yb = np.zeros((64, YBC), np.float32)
        yb[:, 0:512] = y[b]
        yb[:, 512] = mu_b2
        yb[:, 513] = lv_b2
        in_maps.append(
            {
                "w1": w1,
                "w2": np.ascontiguousarray(w2),
                "b1": np.ascontiguousarray(b1),
                "xa": np.ascontiguousarray(x[b, 0:128]),
                "xb": np.ascontiguousarray(x[b, 128:192]),
                "yb": yb,
            }
        )
    return in_maps


def kernel(**inputs) -> np.ndarray:
    from concourse.bass_utils import run_bass_kernel_spmd

    if "nc" not in _CACHE:
        _CACHE["nc"] = build_nc(debug=False)
    nc = _CACHE["nc"]

    in_maps = pack_inputs(inputs)
    res = run_bass_kernel_spmd(nc, in_maps, core_ids=list(range(NCORES)))
    tot = 0.0
    for r in res.results:
        tot += float(r["acc"].reshape(-1)[0])
    loss = -0.5 * tot / (B * L)
    return np.array(loss, dtype=np.float32)


# revision 25
# speedup vs baseline: 1.2139x; 1.0032x over previous
"""CLUB loss kernel for 8x TRN2 NeuronCores.

Math: the reference computes, per sample b (L=512 positions, D=64 dims):
  mu     = MLP_mu(x);  logvar = tanh(MLP_lv(x));  iv = exp(-logvar)
  positive[d,l] = -(mu - y)^2 * 0.5 * iv
  negative[d,l] = -mean_j (y[d,j] - mu[d,l])^2 * 0.5 * iv
  loss = mean over (b,l) of sum_d (positive - negative)

The pairwise LxL mean collapses via moments of y over positions:
  mean_j (y_j - mu)^2 = Ey2 - 2*mu*Ey + mu^2
so with yd2 = 2*(y - Ey), ysq = y^2:
  loss = -0.5/(B*L) * sum_{b,d,l} [ ((ysq - Ey2) - mu*yd2) * iv ].

Sharding: data-parallel over batch B=8, one sample per core; each core
returns a [64,1] per-partition partial, host does the tiny final reduce.

Matmul operands are float32r (fp32 bits, single-pass PE mode, ~4x faster
than fp32 matmul); y, b2 biases and all elementwise math stay fp32.

Input packing per core:
  img [128, 1154] f32r:
    0:128    w1muT_a   = mu_W1.T rows 0:128
    128:256  w1lvT_a   = lv_W1.T rows 0:128
    256:384  w1muT_b   = mu_W1.T rows 128:192 in rows 64:128 (rows 0:64 pad)
    384:512  w1lvT_b   = lv_W1.T rows 128:192 in rows 64:128 (rows 0:64 pad)
    512:576  w2muT     = mu_W2.T
    576:640  w2lvT     = lv_W2.T
    640:642  b1mu, b1lv
    642:1154 xa        = x rows 0:128
  xb [64, 512] f32r: x rows 128:192 -> SBUF partitions 64:128 (aligned with
    the chunk-B weights; matmul requires matching base partitions)
  yb [64, 514] f32: y (cols 0:512), b2mu (col 512), b2lv (col 513)
"""

import sys

if "/opt/trn_rl_repo" not in sys.path:
    sys.path.insert(0, "/opt/trn_rl_repo")

import numpy as np

B, L = 8, 512
XD, YD, H = 192, 64, 128
NCORES = 8
IMGC = 1154
YBC = 514

_CACHE: dict = {}


def build_nc(debug: bool = False, warmup_mms: int = 16):
    import concourse.bass as bass
    import concourse.bacc as bacc
    import concourse.tile as tile
    from concourse import mybir

    f32 = mybir.dt.float32
    f32r = mybir.dt.float32r
    AF = mybir.ActivationFunctionType
    OP = mybir.AluOpType

    nc = bacc.Bacc("TRN2", target_bir_lowering=False, debug=debug)

    w1_d = nc.dram_tensor("w1", [128, 512], f32r, kind="ExternalInput")
    w2_d = nc.dram_tensor("w2", [128, 128], f32r, kind="ExternalInput")
    b1_d = nc.dram_tensor("b1", [128, 2], f32, kind="ExternalInput")
    xa_d = nc.dram_tensor("xa", [128, L], f32r, kind="ExternalInput")
    xb_d = nc.dram_tensor("xb", [64, L], f32r, kind="ExternalInput")
    yb_d = nc.dram_tensor("yb", [64, YBC], f32, kind="ExternalInput")
    acc_d = nc.dram_tensor("acc", [1, 1], f32, kind="ExternalOutput")

    with tile.TileContext(nc) as tc:
        with (
            tc.tile_pool(name="sb", bufs=1) as sb,
            tc.tile_pool(name="ps", bufs=1, space=bass.MemorySpace.PSUM) as ps,
        ):
            ones = sb.tile([64, 1], mybir.dt.float32, tag="ones")
            nc.gpsimd.memset(ones, 1.0)
            # Spread input DMAs over three independent queues so the
            # matmul-gating tensors (w1, xa) each get a full HWDGE ring.
            w1t = sb.tile([128, 512], f32r, tag="w1t")
            nc.sync.dma_start(out=w1t, in_=w1_d[:, :])
            xat = sb.tile([128, L], f32r, tag="xat")
            nc.scalar.dma_start(out=xat, in_=xa_d[:, :])
            ybt = sb.tile([64, YBC], f32, tag="ybt")
            nc.gpsimd.dma_start(out=ybt, in_=yb_d[:, :])
            xbr = sb.tile([128, L], f32r, tag="xbr")
            nc.gpsimd.dma_start(out=xbr[64:128, :], in_=xb_d[:, :])
            w2t = sb.tile([128, 128], f32r, tag="w2t")
            nc.gpsimd.dma_start(out=w2t, in_=w2_d[:, :])
            b1t = sb.tile([128, 2], f32, tag="b1t")
            nc.gpsimd.dma_start(out=b1t, in_=b1_d[:, :])

            # Warm the PE HAM (cold PE runs at 1.2 GHz, warm at 2.4 GHz; the
            # activity monitor needs ~3.4us of sustained work): chain tiny
            # matmuls on the `ones` tile into a scratch PSUM slot while the
            # input DMAs are in flight.
            warm_ps = ps.tile([1, 1], f32, tag="warm")
            for _ in range(warmup_mms):
                nc.tensor.matmul(warm_ps, ones, ones, start=True, stop=True)

            w1muT_a = w1t[:, 0:128]
            w1lvT_a = w1t[:, 128:256]
            w1muT_b = w1t[64:128, 256:384]
            w1lvT_b = w1t[64:128, 384:512]
            w2muT = w2t[:, 0:64]
            w2lvT = w2t[:, 64:128]
            b1mu = b1t[:, 0:1]
            b1lv = b1t[:, 1:2]
            xa = xat[:, :]
            xb = xbr[64:128, :]
            y = ybt[:, 0:512]
            b2mu = ybt[:, 512:513]
            b2lv = ybt[:, 513:514]

            # --- moments of y over L ---
            sumy = sb.tile([64, 1], f32, tag="sumy")
            nc.vector.reduce_sum(out=sumy, in_=y, axis=mybir.AxisListType.X)
            ey = sb.tile([64, 1], f32, tag="ey")
            nc.scalar.mul(ey, sumy, 1.0 / L)
            ysq = sb.tile([64, L], f32, tag="ysq")
            sumy2 = sb.tile([64, 1], f32, tag="sumy2")
            nc.scalar.activation(out=ysq, in_=y, func=AF.Square, accum_out=sumy2)
            ey2 = sb.tile([64, 1], f32, tag="ey2")
            nc.scalar.mul(ey2, sumy2, 1.0 / L)

            # yd2 is needed by both L-halves of the elementwise chain
            yd2 = sb.tile([64, L], f32, tag="yd2")
            nc.vector.tensor_scalar(
                out=yd2, in0=y, scalar1=ey, scalar2=2.0, op0=OP.subtract, op1=OP.mult
            )

            # --- layer 1 (full-width matmuls) ---
            h_mu = ps.tile([128, L], f32, tag="hmu")
            nc.tensor.matmul(h_mu, w1muT_a, xa, start=True, stop=False)
            nc.tensor.matmul(h_mu, w1muT_b, xb, start=False, stop=True)
            h_lv = ps.tile([128, L], f32, tag="hlv")
            nc.tensor.matmul(h_lv, w1lvT_a, xa, start=True, stop=False)
            nc.tensor.matmul(h_lv, w1lvT_b, xb, start=False, stop=True)

            # --- layer 2 + elementwise, chunked over L halves for pipelining ---
            HC = L // 2
            acc_ps = ps.tile([1, 1], f32, tag="accps")
            h_mu_s = sb.tile([128, L], f32r, tag="hmus")
            h_lv_s = sb.tile([128, L], f32r, tag="hlvs")
            for c in range(2):
                cs = slice(c * HC, (c + 1) * HC)
                nc.scalar.activation(
                    out=h_mu_s[:, cs], in_=h_mu[:, cs], func=AF.Relu, bias=b1mu, scale=1.0
                )
                mu_nb = ps.tile([64, HC], f32, tag="munb")
                nc.tensor.matmul(mu_nb, w2muT, h_mu_s[:, cs], start=True, stop=True)
                nc.scalar.activation(
                    out=h_lv_s[:, cs], in_=h_lv[:, cs], func=AF.Relu, bias=b1lv, scale=1.0
                )
                lv_nb = ps.tile([64, HC], f32, tag="lvnb")
                nc.tensor.matmul(lv_nb, w2lvT, h_lv_s[:, cs], start=True, stop=True)
                t1 = sb.tile([64, HC], f32, tag="t1")
                nc.scalar.activation(out=t1, in_=lv_nb, func=AF.Tanh, bias=b2lv, scale=1.0)
                iv = sb.tile([64, HC], f32, tag="iv")
                nc.scalar.activation(out=iv, in_=t1, func=AF.Exp, scale=-1.0)
                w = sb.tile([64, HC], f32, tag="w")
                nc.vector.scalar_tensor_tensor(
                    out=w, in0=mu_nb, scalar=b2mu, in1=yd2[:, cs], op0=OP.add, op1=OP.mult
                )
                v = sb.tile([64, HC], f32, tag="v")
                nc.vector.scalar_tensor_tensor(
                    out=v, in0=ysq[:, cs], scalar=ey2, in1=w,
                    op0=OP.subtract, op1=OP.subtract,
                )
                scr = sb.tile([64, HC], f32, tag="scr")
                acc_s = sb.tile([64, 1], f32, tag="accs")
                nc.vector.scalar_tensor_tensor(
                    out=scr, in0=v, scalar=1.0, in1=iv,
                    op0=OP.mult, op1=OP.mult, accum_out=acc_s,
                )
                # collapse [64,1] partials into one running scalar on the PE
                # (ones.T @ acc) so the output DMA is a single 4-byte packet —
                # a [64,1] store costs 64 tiny descriptors + 16 lazy sem incs.
                nc.tensor.matmul(
                    acc_ps, acc_s, ones, start=(c == 0), stop=(c == 1)
                )
            acc_sb = sb.tile([1, 1], f32, tag="accsb")
            nc.scalar.copy(acc_sb, acc_ps)
            nc.sync.dma_start(out=acc_d[:, :], in_=acc_sb, single_packet=True)

    nc.compile()
    return nc


def pack_inputs(inputs: dict) -> list[dict]:
    x = np.ascontiguousarray(np.asarray(inputs["x_samples"], dtype=np.float32))
    y = np.ascontiguousarray(np.asarray(inputs["y_samples"], dtype=np.float32))
    mu_W1 = np.asarray(inputs["mu_W1"], dtype=np.float32)
    mu_b1 = np.asarray(inputs["mu_b1"], dtype=np.float32)
    mu_W2 = np.asarray(inputs["mu_W2"], dtype=np.float32)
    mu_b2 = np.asarray(inputs["mu_b2"], dtype=np.float32)
    lv_W1 = np.asarray(inputs["lv_W1"], dtype=np.float32)
    lv_b1 = np.asarray(inputs["lv_b1"], dtype=np.float32)
    lv_W2 = np.asarray(inputs["lv_W2"], dtype=np.float32)
    lv_b2 = np.asarray(inputs["lv_b2"], dtype=np.float32)

    w1 = np.zeros((128, 512), np.float32)
    w1muT = mu_W1.T  # [192, 128]
    w1lvT = lv_W1.T
    w1[:, 0:128] = w1muT[0:128]
    w1[:, 128:256] = w1lvT[0:128]
    w1[64:128, 256:384] = w1muT[128:192]
    w1[64:128, 384:512] = w1lvT[128:192]
    w2 = np.concatenate([mu_W2.T, lv_W2.T], axis=1)  # [128, 128]
    b1 = np.stack([mu_b1, lv_b1], axis=1)  # [128, 2]

    in_maps = []
    for b in range(NCORES):
        yb = np.zeros((64, YBC), np.float32)
        yb[:, 0:512] = y[b]
        yb[:, 512] = mu_b2
        yb[:, 513] = lv_b2
        in_maps.append(
            {
                "w1": w1,
                "w2": np.ascontiguousarray(w2),
                "b1": np.ascontiguousarray(b1),
                "xa": np.ascontiguousarray(x[b, 0:128]),
                "xb": np.ascontiguousarray(x[b, 128:192]),
                "yb": yb,
            }
        )
    return in_maps


def kernel(**inputs) -> np.ndarray:
    from concourse.bass_utils import run_bass_kernel_spmd

    if "nc" not in _CACHE:
        _CACHE["nc"] = build_nc(debug=False)
    nc = _CACHE["nc"]

    in_maps = pack_inputs(inputs)
    res = run_bass_kernel_spmd(nc, in_maps, core_ids=list(range(NCORES)))
    tot = 0.0
    for r in res.results:
        tot += float(r["acc"].reshape(-1)[0])
    loss = -0.5 * tot / (B * L)
    return np.array(loss, dtype=np.float32)
